# revision 1
# baseline (speedup 1.0000x reference)
"""Trainium2 Bass kernel for nn_ConnectLoss (pairwise BCE-Dice instance loss).

Strategy (8 NeuronCores, pixel-sharded; pixels = contraction dim of a joint
label histogram computed on the TensorE):
  - Each core gets H/8 = 256 rows (524288 pixels) of all four inputs.
  - Stationary operand per 128-pixel block: target one-hots toh [128, 16]
    (fp16, DVE is_equal planes). Moving operand: 36 columns = 31 pred one-hot
    planes + [ones, cls, ln(cls), ln(1-cls), ps^2]. PSUM accumulates
    out[16, 36] = all pairwise counts + per-target-class auxiliary sums.
  - PE column-tiling (128x32 mode): consecutive pixel blocks rotate over the
    4 independent column tiles so their matmuls overlap in the array; the 4
    partial accumulators are summed on-device at the end.
  - Pred one-hot planes split between DVE (is_equal, 4x mode) and ACT
    (relu(1-(pm-k)^2), exact for integer labels) to balance engine load.
  - Output is a single [16, 36] f32 tile per core; final min/sum on host.

cls_out is uniform in [1e-4, 1-1e-4] so the torch-style -100 log clamp can
never trigger; logs are computed unclamped from the f32 tile (fp16 would
round cls to 1.0 and produce -inf in ln(1-cls)).
"""

import sys

if "/opt/trn_rl_repo" not in sys.path:
    sys.path.insert(0, "/opt/trn_rl_repo")

import numpy as np
from contextlib import ExitStack

# ---------------------------------------------------------------- constants
P = 128
H, W = 2048, 2048
NCORES = 8
ROWS = H // NCORES                 # 256 rows per core
PIX = ROWS * W                     # 524288 pixels per core
FPP = PIX // P                     # 4096 pixel columns per partition
CF = 1024                          # chunk free size
NCHUNK = FPP // CF                 # 4 chunks
SUBF = 512                         # toh subchunk free size
K = 32                             # pred instance classes
KB = K - 1                         # pred one-hot planes built (k=31 derived)
DVEK = 28                          # pred planes built on DVE (k < DVEK)
N = 16                             # target instance classes
MCOL = KB + 5                      # 36 moving cols: poh + ones,cls,lnp,ln1mp,ps2
C_ONE = KB                         # ones column index
C_CLS = KB + 1
C_LNP = KB + 2
C_LN1MP = KB + 3
C_PS2 = KB + 4
NTILE = 4                          # PE column tiles (128x32 mode)

SMOOTH = 1.0
HWPIX = float(H * W)

_cached = {}


def _build_bass():
    import concourse.bass as bass
    import concourse.bacc as bacc
    import concourse.mybir as mybir
    from concourse.tile import TileContext

    f32 = mybir.dt.float32
    f16 = mybir.dt.float16
    i32 = mybir.dt.int32
    eq = mybir.AluOpType.is_equal
    add = mybir.AluOpType.add
    AF = mybir.ActivationFunctionType

    nc = bacc.Bacc("TRN2", num_swdge_queues=4)
    pm_d = nc.dram_tensor("pm", [PIX], i32, kind="ExternalInput")
    tm_d = nc.dram_tensor("tm", [PIX], i32, kind="ExternalInput")
    cls_d = nc.dram_tensor("cls", [PIX], f32, kind="ExternalInput")
    ps_d = nc.dram_tensor("ps", [PIX], f32, kind="ExternalInput")
    out_d = nc.dram_tensor("out", [N, MCOL], f32, kind="ExternalOutput")

    pm_v = pm_d[:].rearrange("(p f) -> p f", p=P)
    tm_v = tm_d[:].rearrange("(p f) -> p f", p=P)
    cls_v = cls_d[:].rearrange("(p f) -> p f", p=P)
    ps_v = ps_d[:].rearrange("(p f) -> p f", p=P)

    with ExitStack() as es:
        tc = es.enter_context(TileContext(nc))
        pool_in = es.enter_context(tc.tile_pool(name="inp", bufs=2))
        pool_toh = es.enter_context(tc.tile_pool(name="toh", bufs=2))
        pool_poh = es.enter_context(tc.tile_pool(name="poh", bufs=2))
        pool_sq = es.enter_context(tc.tile_pool(name="sq", bufs=2))
        pool_misc = es.enter_context(tc.tile_pool(name="misc", bufs=1))
        psum = es.enter_context(tc.tile_pool(name="ps", bufs=1, space="PSUM"))

        inter_ps = psum.tile([P, MCOL], f32)

        # per-k bias columns for the ACT-built one-hot planes
        bias_k = {}
        for k in range(DVEK, KB):
            t = pool_misc.tile([P, 1], f32, tag=f"bk{k}")
            nc.gpsimd.memset(t[:], -float(k))
            bias_k[k] = t

        for c in range(NCHUNK):
            cs = slice(c * CF, (c + 1) * CF)
            pm16 = pool_in.tile([P, CF], f16, tag="pm16")
            tm16 = pool_in.tile([P, CF], f16, tag="tm16")
            ps16 = pool_in.tile([P, CF], f16, tag="ps16")
            cls_t = pool_in.tile([P, CF], f32, tag="cls")
            nc.gpsimd.dma_start(out=pm16[:], in_=pm_v[:, cs])
            nc.gpsimd.dma_start(out=tm16[:], in_=tm_v[:, cs])
            nc.gpsimd.dma_start(out=ps16[:], in_=ps_v[:, cs])
            nc.sync.dma_start(out=cls_t[:], in_=cls_v[:, cs])

            pohv = pool_poh.tile([P, MCOL * CF], f16, tag="pohv")

            def col(i):
                return pohv[:, i * CF:(i + 1) * CF]

            # pred one-hots: k < DVEK on DVE (is_equal), rest on ACT via
            # relu(1 - (pm-k)^2) — exact for integer labels
            for k in range(DVEK):
                nc.vector.tensor_scalar(col(k), pm16[:], float(k), None, eq)
            for k in range(DVEK, KB):
                sq = pool_sq.tile([P, CF], f16, tag="sq")
                nc.scalar.activation(sq[:], pm16[:], AF.Square, bias=bias_k[k][:])
                nc.scalar.activation(col(k), sq[:], AF.Relu, bias=1.0, scale=-1.0)
            nc.gpsimd.memset(col(C_ONE), 1.0)
            nc.gpsimd.dma_start(out=col(C_CLS), in_=cls_v[:, cs])  # f32->f16
            nc.scalar.activation(col(C_LNP), cls_t[:], AF.Ln)
            nc.scalar.activation(col(C_LN1MP), cls_t[:], AF.Ln, bias=1.0, scale=-1.0)
            nc.scalar.activation(col(C_PS2), ps16[:], AF.Square)

            poh3 = pohv[:].rearrange("p (m f) -> p f m", m=MCOL)
            for s in range(CF // SUBF):
                soff = s * SUBF
                toh = pool_toh.tile([P, N * SUBF], f16, tag="toh")
                for n in range(N):
                    nc.vector.tensor_scalar(
                        toh[:, n * SUBF:(n + 1) * SUBF],
                        tm16[:, soff:soff + SUBF], float(n), None, eq,
                    )
                toh3 = toh[:].rearrange("p (n f) -> p f n", n=N)
                for jj in range(SUBF):
                    j = c * CF + soff + jj
                    g = j % NTILE
                    nc.tensor.matmul(
                        inter_ps[32 * g:32 * g + N, :],
                        toh3[:, jj:jj + 1, :],
                        poh3[:, soff + jj:soff + jj + 1, :],
                        start=(j < NTILE),
                        stop=(j >= FPP - NTILE),
                        tile_position=(0, 32 * g),
                        skip_group_check=True,
                    )

        # evacuate the column-tile accumulators and sum them
        accf = pool_misc.tile([N, MCOL], f32)
        if NTILE == 1:
            nc.vector.tensor_copy(accf[:], inter_ps[0:N, :])
        else:
            sb = pool_misc.tile([P, MCOL], f32)
            for g in range(NTILE):
                nc.vector.tensor_copy(
                    sb[32 * g:32 * g + N, :], inter_ps[32 * g:32 * g + N, :]
                )
            sh = [
                pool_misc.tile([N, MCOL], f32, tag=f"sh{g}", name=f"sh{g}")
                for g in range(NTILE - 1)
            ]
            for g in range(1, NTILE):
                nc.sync.dma_start(out=sh[g - 1][:], in_=sb[32 * g:32 * g + N, :])
            acc01 = pool_misc.tile([N, MCOL], f32)
            nc.vector.tensor_tensor(acc01[:], sb[0:N, :], sh[0][:], add)
            if NTILE == 4:
                acc23 = pool_misc.tile([N, MCOL], f32)
                nc.vector.tensor_tensor(acc23[:], sh[1][:], sh[2][:], add)
                nc.vector.tensor_tensor(accf[:], acc01[:], acc23[:], add)
            else:
                nc.vector.tensor_copy(accf[:], acc01[:])
        nc.scalar.dma_start(out=out_d[:, :], in_=accf[:])

    nc.finalize()
    return nc


def _get_nc():
    if "nc" not in _cached:
        _cached["nc"] = _build_bass()
    return _cached["nc"]


def _get_runner():
    """Build the sharded jitted executable ONCE; reuse across calls."""
    if "runner" in _cached:
        return _cached["runner"]

    import jax
    import concourse.mybir as mybir
    from jax.sharding import Mesh, PartitionSpec
    from jax.experimental.shard_map import shard_map
    from concourse import bass2jax

    bass2jax.install_neuronx_cc_hook()
    nc = _get_nc()
    partition_name = (
        nc.partition_id_tensor.name if nc.partition_id_tensor else None
    )

    in_names, out_names, out_avals, zero_outs = [], [], [], []
    for alloc in nc.m.functions[0].allocations:
        if not isinstance(alloc, mybir.MemoryLocationSet):
            continue
        name = alloc.memorylocations[0].name
        if alloc.kind == "ExternalInput":
            if name != partition_name:
                in_names.append(name)
        elif alloc.kind == "ExternalOutput":
            out_names.append(name)
            shape = tuple(alloc.tensor_shape)
            dtype = mybir.dt.np(alloc.dtype)
            out_avals.append(jax.core.ShapedArray(shape, dtype))
            zero_outs.append(np.zeros(shape, dtype))
    n_params = len(in_names)
    n_outs = len(out_avals)
    all_in_names = list(in_names) + list(out_names)
    if partition_name is not None:
        all_in_names.append(partition_name)
    donate = tuple(range(n_params, n_params + n_outs))

    def _body(*args):
        operands = list(args)
        if partition_name is not None:
            operands.append(bass2jax.partition_id_tensor())
        outs = bass2jax._bass_exec_p.bind(
            *operands,
            out_avals=tuple(out_avals),
            in_names=tuple(all_in_names),
            out_names=tuple(out_names),
            lowering_input_output_aliases=(),
            sim_require_finite=True,
            sim_require_nnan=True,
            nc=nc,
        )
        return tuple(outs)

    devices = jax.devices()[:NCORES]
    mesh = Mesh(np.asarray(devices), ("core",))
    in_specs = (PartitionSpec("core"),) * (n_params + n_outs)
    out_specs = (PartitionSpec("core"),) * n_outs
    sharded = jax.jit(
        shard_map(
            _body, mesh=mesh, in_specs=in_specs, out_specs=out_specs,
            check_rep=False,
        ),
        donate_argnums=donate,
        keep_unused=True,
    )

    def run(in_maps):
        if isinstance(in_maps, dict):
            # fast path: full flattened arrays (row-sharding = identity here)
            concat_in = [in_maps[name] for name in in_names]
        else:
            concat_in = [
                np.concatenate([np.asarray(m[name]) for m in in_maps], axis=0)
                for name in in_names
            ]
        concat_zeros = [
            np.zeros((NCORES * z.shape[0], *z.shape[1:]), z.dtype)
            for z in zero_outs
        ]
        out_arrs = sharded(*concat_in, *concat_zeros)
        return [
            {
                name: np.asarray(out_arrs[i]).reshape(
                    NCORES, *out_avals[i].shape)[c]
                for i, name in enumerate(out_names)
            }
            for c in range(NCORES)
        ]

    def bench(in_maps, iters=20):
        """Time the sharded call with device-resident inputs."""
        import time
        from jax.sharding import NamedSharding

        concat_in = [
            np.concatenate([np.asarray(m[name]) for m in in_maps], axis=0)
            for name in in_names
        ]
        shard = NamedSharding(mesh, PartitionSpec("core"))
        dev_in = [jax.device_put(x, shard) for x in concat_in]
        zeros = [
            np.zeros((NCORES * z.shape[0], *z.shape[1:]), z.dtype)
            for z in zero_outs
        ]

        def call():
            zs = [jax.device_put(z, shard) for z in zeros]
            outs = sharded(*dev_in, *zs)
            for o in outs:
                o.block_until_ready()

        call()
        ts = []
        for _ in range(iters):
            t0 = time.perf_counter()
            call()
            ts.append(time.perf_counter() - t0)
        return min(ts), sum(ts) / len(ts)

    run.bench = bench
    _cached["runner"] = run
    return run


def kernel(pred_instance_mask, pred_score, cls_out, target_mask):
    run = _get_runner()

    # row-sharding across cores: the concatenation of per-core row slices is
    # just the full array flattened, so pass views of the full inputs
    full = {
        "pm": np.ascontiguousarray(pred_instance_mask).reshape(-1).astype(
            np.int32, copy=False),
        "tm": np.ascontiguousarray(target_mask).reshape(-1).astype(
            np.int32, copy=False),
        "cls": np.ascontiguousarray(cls_out).reshape(-1).astype(
            np.float32, copy=False),
        "ps": np.ascontiguousarray(pred_score).reshape(-1).astype(
            np.float32, copy=False),
    }

    outs = [r["out"] for r in run(full)]

    acc = np.zeros((N, MCOL), dtype=np.float64)
    for o in outs:
        acc += o.astype(np.float64)
    return _host_finish(acc)


def _host_finish(acc):
    inter31 = acc[:, 0:KB]
    st = acc[:, C_ONE]                       # [N] target marginals
    inter = np.concatenate(
        [inter31, (st - inter31.sum(axis=1))[:, None]], axis=1)
    sp = inter.sum(axis=0)                   # [K] pred marginals

    sum_t = HWPIX - st[0]                    # count(target > 0)
    sum_p = acc[:, C_CLS].sum()              # sum(cls_out)
    sum_logp = acc[:, C_LNP].sum()
    inter_cls = sum_p - acc[0, C_CLS]        # sum over target>0 of cls_out
    bce_sum = (sum_logp - acc[0, C_LNP]) + acc[0, C_LN1MP]
    ps2 = acc[:, C_PS2].sum()

    mse = ps2 / HWPIX
    bce_cls = -bce_sum / HWPIX
    dice_cls = 1.0 - (2.0 * inter_cls + SMOOTH) / (sum_p + sum_t + SMOOTH)

    union = st[:, None] + sp[None, :]
    bce_pair = 100.0 * (union - 2.0 * inter) / HWPIX
    dice_pair = 1.0 - (2.0 * inter + SMOOTH) / (union + SMOOTH)
    pair = bce_pair + dice_pair
    res = mse + bce_cls + dice_cls + pair.min(axis=1).sum()
    return np.float32(res / float(N))



# revision 3
# speedup vs baseline: 2.1145x; 2.1145x over previous
"""Trainium2 Bass kernel for nn_ConnectLoss (pairwise BCE-Dice instance loss).

v2 strategy (8 NeuronCores, pixel-sharded; pixels = contraction dim of a joint
label histogram computed on the TensorE):
  - Each core gets H/8 = 256 rows (524288 pixels = [128 lanes, 4096]) of all
    four inputs, processed in 4 chunks of CF=1024 pixel-columns.
  - Histogram matmul, grouped G=4 blocks per instruction on two concurrent
    64-column PE tiles: stationary = target one-hots for 4 blocks, laid out
    interleaved [g, n, b] so each group's 64 columns are contiguous (DVE
    builds them at 4x with 4-element-run output APs); moving = 36 plane-major
    columns x 4-block runs (144 cols/MM). True results live on the b-diagonal
    of two persistent PSUM banks; off-diagonal products are ignored.
  - Moving planes: 31 pred one-hots (24 on DVE is_equal @4x bf16, 7 on ACT
    via relu(1-(pm-k)^2)), ones (memset), cls (cast DMA), ln(cls),
    ln(1-cls), ps^2 (ACT).
  - Everything bf16 (integer labels < 512 exact; DVE 4x mode needs 16-bit).
  - Output: [128, 144] f32 per core (2 banks x 64 rows); host extracts the
    block-diagonal, reduces to the same [16, 36] stats layout as v1, and
    finishes the tiny min/sum on the host.
"""

import sys

if "/opt/trn_rl_repo" not in sys.path:
    sys.path.insert(0, "/opt/trn_rl_repo")

import numpy as np
from contextlib import ExitStack

# ---------------------------------------------------------------- constants
P = 128
H, W = 2048, 2048
NCORES = 8
ROWS = H // NCORES                 # 256 rows per core
PIX = ROWS * W                     # 524288 pixels per core
FPP = PIX // P                     # 4096 pixel columns per partition
CF = 1024                          # chunk free size
NCHUNK = FPP // CF                 # 4 chunks
SUBF = 512                         # toh half-chunk free size
GB = 4                             # blocks per matmul group
K = 32                             # pred instance classes
KB = K - 1                         # pred one-hot planes built (k=31 derived)
DVEK = 24                          # pred planes built on DVE (k < DVEK)
N = 16                             # target instance classes
MCOL = KB + 5                      # 36 moving planes
C_ONE = KB                         # ones plane index
C_CLS = KB + 1
C_LNP = KB + 2
C_LN1MP = KB + 3
C_PS2 = KB + 4
MMF = MCOL * GB                    # 144 moving cols per grouped matmul

SMOOTH = 1.0
HWPIX = float(H * W)

_cached = {}


def _build_bass():
    import concourse.bass as bass
    import concourse.bacc as bacc
    import concourse.mybir as mybir
    from concourse.tile import TileContext

    f32 = mybir.dt.float32
    bf16 = mybir.dt.bfloat16
    i32 = mybir.dt.int32
    eq = mybir.AluOpType.is_equal
    AF = mybir.ActivationFunctionType

    nc = bacc.Bacc("TRN2", num_swdge_queues=4)
    pm_d = nc.dram_tensor("pm", [PIX], i32, kind="ExternalInput")
    tm_d = nc.dram_tensor("tm", [PIX], i32, kind="ExternalInput")
    cls_d = nc.dram_tensor("cls", [PIX], f32, kind="ExternalInput")
    ps_d = nc.dram_tensor("ps", [PIX], f32, kind="ExternalInput")
    out_d = nc.dram_tensor("out", [P, MMF], f32, kind="ExternalOutput")

    pm_v = pm_d[:].rearrange("(p f) -> p f", p=P)
    tm_v = tm_d[:].rearrange("(p f) -> p f", p=P)
    cls_v = cls_d[:].rearrange("(p f) -> p f", p=P)
    ps_v = ps_d[:].rearrange("(p f) -> p f", p=P)

    NGRP = CF // GB                # 256 matmul groups per chunk
    NGH = SUBF // GB               # 128 groups per toh half-tile

    with ExitStack() as es:
        tc = es.enter_context(TileContext(nc))
        pool_in = es.enter_context(tc.tile_pool(name="inp", bufs=2))
        pool_toh = es.enter_context(tc.tile_pool(name="toh", bufs=2))
        pool_poh = es.enter_context(tc.tile_pool(name="poh", bufs=2))
        pool_sq = es.enter_context(tc.tile_pool(name="sq", bufs=2))
        pool_misc = es.enter_context(tc.tile_pool(name="misc", bufs=1))
        psum = es.enter_context(tc.tile_pool(name="ps", bufs=1, space="PSUM"))

        banks = [psum.tile([P, MMF], f32, tag=f"bk{t}", name=f"bk{t}")
                 for t in range(2)]

        # per-k bias columns for the ACT-built one-hot planes
        bias_k = {}
        for k in range(DVEK, KB):
            t = pool_misc.tile([P, 1], f32, tag=f"bk{k}")
            nc.gpsimd.memset(t[:], -float(k))
            bias_k[k] = t

        for c in range(NCHUNK):
            cs = slice(c * CF, (c + 1) * CF)
            pm16 = pool_in.tile([P, CF], bf16, tag="pm16")
            tm16 = pool_in.tile([P, CF], bf16, tag="tm16")
            ps16 = pool_in.tile([P, CF], bf16, tag="ps16")
            clsf = pool_in.tile([P, CF], f32, tag="clsf")
            nc.gpsimd.dma_start(out=pm16[:], in_=pm_v[:, cs])
            nc.gpsimd.dma_start(out=tm16[:], in_=tm_v[:, cs])
            nc.gpsimd.dma_start(out=ps16[:], in_=ps_v[:, cs])       # f32->bf16
            nc.sync.dma_start(out=clsf[:], in_=cls_v[:, cs])        # raw f32

            pohv = pool_poh.tile([P, MCOL * CF], bf16, tag="pohv")

            def col(i):
                return pohv[:, i * CF:(i + 1) * CF]

            # aux planes: ones, cls, ln(cls), ln(1-cls), ps^2.  The ln ops
            # MUST read f32 cls: bf16 rounds cls>0.998 to 1.0 -> ln(0).
            nc.gpsimd.memset(col(C_ONE), 1.0)
            nc.gpsimd.dma_start(out=col(C_CLS), in_=clsf[:])        # f32->bf16
            nc.scalar.activation(col(C_LNP), clsf[:], AF.Ln)
            nc.scalar.activation(col(C_LN1MP), clsf[:], AF.Ln,
                                 bias=1.0, scale=-1.0)
            nc.scalar.activation(col(C_PS2), ps16[:], AF.Square)

            # pred one-hots: k < DVEK on DVE (is_equal @4x), rest on ACT via
            # relu(1-(pm-k)^2) -- exact for integer labels
            for k in range(DVEK):
                nc.vector.tensor_scalar(col(k), pm16[:], float(k), None, eq)
            for k in range(DVEK, KB):
                sq = pool_sq.tile([P, CF], bf16, tag="sq")
                nc.scalar.activation(sq[:], pm16[:], AF.Square, bias=bias_k[k][:])
                nc.scalar.activation(col(k), sq[:], AF.Relu, bias=1.0, scale=-1.0)

            pohv_kb = pohv[:].rearrange("p (m f) -> p m f", m=MCOL)

            for s in range(CF // SUBF):
                # target one-hots, 4-run interleaved layout [g, n, b]
                toh = pool_toh.tile([P, N * SUBF], bf16, tag="toh")
                toh4 = toh[:].rearrange("p (g n b) -> p g n b", n=N, b=GB)
                soff = s * SUBF
                for n in range(N):
                    nc.vector.tensor_scalar(
                        toh4[:, :, n, :],
                        tm16[:, soff:soff + SUBF], float(n), None, eq,
                    )
                for gg in range(NGH):
                    g = s * NGH + gg            # group within chunk
                    jg = c * NGRP + g           # global group index
                    t = jg % 2
                    nc.tensor.matmul(
                        banks[t][64 * t:64 * t + 64, :],
                        toh[:, 64 * gg:64 * gg + 64],
                        pohv_kb[:, :, GB * g:GB * g + GB],
                        start=(jg < 2),
                        stop=(jg >= NCHUNK * NGRP - 2),
                        tile_position=(0, 64 * t),
                        skip_group_check=True,
                    )

        # evacuate the two psum banks and store [128, 144] f32
        ev = pool_misc.tile([P, MMF], f32)
        nc.vector.tensor_copy(ev[0:64, :], banks[0][0:64, :])
        nc.vector.tensor_copy(ev[64:128, :], banks[1][64:128, :])
        nc.sync.dma_start(out=out_d[:, :], in_=ev[:])

    nc.finalize()
    return nc


def _get_nc():
    if "nc" not in _cached:
        _cached["nc"] = _build_bass()
    return _cached["nc"]


def _get_runner():
    """Build the sharded jitted executable ONCE; reuse across calls."""
    if "runner" in _cached:
        return _cached["runner"]

    import jax
    import concourse.mybir as mybir
    from jax.sharding import Mesh, PartitionSpec
    from jax.experimental.shard_map import shard_map
    from concourse import bass2jax

    bass2jax.install_neuronx_cc_hook()
    nc = _get_nc()
    partition_name = (
        nc.partition_id_tensor.name if nc.partition_id_tensor else None
    )

    in_names, out_names, out_avals, zero_outs = [], [], [], []
    for alloc in nc.m.functions[0].allocations:
        if not isinstance(alloc, mybir.MemoryLocationSet):
            continue
        name = alloc.memorylocations[0].name
        if alloc.kind == "ExternalInput":
            if name != partition_name:
                in_names.append(name)
        elif alloc.kind == "ExternalOutput":
            out_names.append(name)
            shape = tuple(alloc.tensor_shape)
            dtype = mybir.dt.np(alloc.dtype)
            out_avals.append(jax.core.ShapedArray(shape, dtype))
            zero_outs.append(np.zeros(shape, dtype))
    n_params = len(in_names)
    n_outs = len(out_avals)
    all_in_names = list(in_names) + list(out_names)
    if partition_name is not None:
        all_in_names.append(partition_name)
    donate = tuple(range(n_params, n_params + n_outs))

    def _body(*args):
        operands = list(args)
        if partition_name is not None:
            operands.append(bass2jax.partition_id_tensor())
        outs = bass2jax._bass_exec_p.bind(
            *operands,
            out_avals=tuple(out_avals),
            in_names=tuple(all_in_names),
            out_names=tuple(out_names),
            lowering_input_output_aliases=(),
            sim_require_finite=True,
            sim_require_nnan=True,
            nc=nc,
        )
        return tuple(outs)

    devices = jax.devices()[:NCORES]
    mesh = Mesh(np.asarray(devices), ("core",))
    in_specs = (PartitionSpec("core"),) * (n_params + n_outs)
    out_specs = (PartitionSpec("core"),) * n_outs
    sharded = jax.jit(
        shard_map(
            _body, mesh=mesh, in_specs=in_specs, out_specs=out_specs,
            check_rep=False,
        ),
        donate_argnums=donate,
        keep_unused=True,
    )

    def run(in_maps):
        if isinstance(in_maps, dict):
            # fast path: full flattened arrays (row-sharding = identity here)
            concat_in = [in_maps[name] for name in in_names]
        else:
            concat_in = [
                np.concatenate([np.asarray(m[name]) for m in in_maps], axis=0)
                for name in in_names
            ]
        concat_zeros = [
            np.zeros((NCORES * z.shape[0], *z.shape[1:]), z.dtype)
            for z in zero_outs
        ]
        out_arrs = sharded(*concat_in, *concat_zeros)
        return [
            {
                name: np.asarray(out_arrs[i]).reshape(
                    NCORES, *out_avals[i].shape)[c]
                for i, name in enumerate(out_names)
            }
            for c in range(NCORES)
        ]

    def bench(in_maps, iters=20):
        """Time the sharded call with device-resident inputs."""
        import time
        from jax.sharding import NamedSharding

        concat_in = [
            np.concatenate([np.asarray(m[name]) for m in in_maps], axis=0)
            for name in in_names
        ]
        shard = NamedSharding(mesh, PartitionSpec("core"))
        dev_in = [jax.device_put(x, shard) for x in concat_in]
        zeros = [
            np.zeros((NCORES * z.shape[0], *z.shape[1:]), z.dtype)
            for z in zero_outs
        ]

        def call():
            zs = [jax.device_put(z, shard) for z in zeros]
            outs = sharded(*dev_in, *zs)
            for o in outs:
                o.block_until_ready()

        call()
        ts = []
        for _ in range(iters):
            t0 = time.perf_counter()
            call()
            ts.append(time.perf_counter() - t0)
        return min(ts), sum(ts) / len(ts)

    run.bench = bench
    _cached["runner"] = run
    return run


def _reduce_core_out(ev):
    """[128, 144] f32 bank dump -> [16, 36] stats (diagonal extraction)."""
    acc = np.zeros((N, MCOL), dtype=np.float64)
    ev = ev.astype(np.float64)
    for t in range(2):
        rows = ev[64 * t:64 * t + 64].reshape(N, GB, MCOL, GB)  # [n, b, k, b']
        for b in range(GB):
            acc += rows[:, b, :, b]
    return acc


def kernel(pred_instance_mask, pred_score, cls_out, target_mask):
    run = _get_runner()

    # row-sharding across cores: the concatenation of per-core row slices is
    # just the full array flattened, so pass views of the full inputs
    full = {
        "pm": np.ascontiguousarray(pred_instance_mask).reshape(-1).astype(
            np.int32, copy=False),
        "tm": np.ascontiguousarray(target_mask).reshape(-1).astype(
            np.int32, copy=False),
        "cls": np.ascontiguousarray(cls_out).reshape(-1).astype(
            np.float32, copy=False),
        "ps": np.ascontiguousarray(pred_score).reshape(-1).astype(
            np.float32, copy=False),
    }

    outs = [r["out"] for r in run(full)]

    acc = np.zeros((N, MCOL), dtype=np.float64)
    for o in outs:
        acc += _reduce_core_out(o)
    return _host_finish(acc)


def _host_finish(acc):
    inter31 = acc[:, 0:KB]
    st = acc[:, C_ONE]                       # [N] target marginals
    inter = np.concatenate(
        [inter31, (st - inter31.sum(axis=1))[:, None]], axis=1)
    sp = inter.sum(axis=0)                   # [K] pred marginals

    sum_t = HWPIX - st[0]                    # count(target > 0)
    sum_p = acc[:, C_CLS].sum()              # sum(cls_out)
    sum_logp = acc[:, C_LNP].sum()
    inter_cls = sum_p - acc[0, C_CLS]        # sum over target>0 of cls_out
    bce_sum = (sum_logp - acc[0, C_LNP]) + acc[0, C_LN1MP]
    ps2 = acc[:, C_PS2].sum()

    mse = ps2 / HWPIX
    bce_cls = -bce_sum / HWPIX
    dice_cls = 1.0 - (2.0 * inter_cls + SMOOTH) / (sum_p + sum_t + SMOOTH)

    union = st[:, None] + sp[None, :]
    bce_pair = 100.0 * (union - 2.0 * inter) / HWPIX
    dice_pair = 1.0 - (2.0 * inter + SMOOTH) / (union + SMOOTH)
    pair = bce_pair + dice_pair
    res = mse + bce_cls + dice_cls + pair.min(axis=1).sum()
    return np.float32(res / float(N))


# revision 5
# speedup vs baseline: 2.1459x; 1.0148x over previous
"""Trainium2 Bass kernel for nn_ConnectLoss (pairwise BCE-Dice instance loss).

v2 strategy (8 NeuronCores, pixel-sharded; pixels = contraction dim of a joint
label histogram computed on the TensorE):
  - Each core gets H/8 = 256 rows (524288 pixels = [128 lanes, 4096]) of all
    four inputs, processed in 4 chunks of CF=1024 pixel-columns.
  - Histogram matmul, grouped G=4 blocks per instruction on two concurrent
    64-column PE tiles: stationary = target one-hots for 4 blocks, laid out
    interleaved [g, n, b] so each group's 64 columns are contiguous (DVE
    builds them at 4x with 4-element-run output APs); moving = 36 plane-major
    columns x 4-block runs (144 cols/MM). True results live on the b-diagonal
    of two persistent PSUM banks; off-diagonal products are ignored.
  - Moving planes: 31 pred one-hots (24 on DVE is_equal @4x bf16, 7 on ACT
    via relu(1-(pm-k)^2)), ones (memset), cls (cast DMA), ln(cls),
    ln(1-cls), ps^2 (ACT).
  - Everything bf16 (integer labels < 512 exact; DVE 4x mode needs 16-bit).
  - Output: [128, 144] f32 per core (2 banks x 64 rows); host extracts the
    block-diagonal, reduces to the same [16, 36] stats layout as v1, and
    finishes the tiny min/sum on the host.
"""

import sys

if "/opt/trn_rl_repo" not in sys.path:
    sys.path.insert(0, "/opt/trn_rl_repo")

import numpy as np
from contextlib import ExitStack

# ---------------------------------------------------------------- constants
P = 128
H, W = 2048, 2048
NCORES = 8
ROWS = H // NCORES                 # 256 rows per core
PIX = ROWS * W                     # 524288 pixels per core
FPP = PIX // P                     # 4096 pixel columns per partition
CF = 1024                          # chunk free size
NCHUNK = FPP // CF                 # 4 chunks
SUBF = 1024                        # toh tile free size (= CF)
GB = 4                             # blocks per matmul group
K = 32                             # pred instance classes
KB = K - 1                         # pred one-hot planes built (k=31 derived)
DVEK = 26                          # pred planes built on DVE (k < DVEK)
N = 16                             # target instance classes
MCOL = KB + 5                      # 36 moving planes
C_ONE = KB                         # ones plane index
C_CLS = KB + 1
C_LNP = KB + 2
C_LN1MP = KB + 3
C_PS2 = KB + 4
MMF = MCOL * GB                    # 144 moving cols per grouped matmul

SMOOTH = 1.0
HWPIX = float(H * W)

_cached = {}


def _build_bass():
    import concourse.bass as bass
    import concourse.bacc as bacc
    import concourse.mybir as mybir
    from concourse.tile import TileContext

    f32 = mybir.dt.float32
    bf16 = mybir.dt.bfloat16
    i32 = mybir.dt.int32
    eq = mybir.AluOpType.is_equal
    _ = mybir  # used for AluOpType below
    AF = mybir.ActivationFunctionType

    nc = bacc.Bacc("TRN2", num_swdge_queues=4)
    pm_d = nc.dram_tensor("pm", [PIX], i32, kind="ExternalInput")
    tm_d = nc.dram_tensor("tm", [PIX], i32, kind="ExternalInput")
    cls_d = nc.dram_tensor("cls", [PIX], f32, kind="ExternalInput")
    ps_d = nc.dram_tensor("ps", [PIX], f32, kind="ExternalInput")
    out_d = nc.dram_tensor("out", [P, MMF], f32, kind="ExternalOutput")

    pm_v = pm_d[:].rearrange("(p f) -> p f", p=P)
    tm_v = tm_d[:].rearrange("(p f) -> p f", p=P)
    cls_v = cls_d[:].rearrange("(p f) -> p f", p=P)
    ps_v = ps_d[:].rearrange("(p f) -> p f", p=P)

    NGRP = CF // GB                # 256 matmul groups per chunk
    NGH = SUBF // GB               # 128 groups per toh half-tile

    with ExitStack() as es:
        tc = es.enter_context(TileContext(nc))
        pool_in = es.enter_context(tc.tile_pool(name="inp", bufs=2))
        pool_toh = es.enter_context(tc.tile_pool(name="toh", bufs=1))
        pool_poh = es.enter_context(tc.tile_pool(name="poh", bufs=2))
        pool_sq = es.enter_context(tc.tile_pool(name="sq", bufs=2))
        pool_misc = es.enter_context(tc.tile_pool(name="misc", bufs=1))
        psum = es.enter_context(tc.tile_pool(name="ps", bufs=1, space="PSUM"))

        banks = [psum.tile([P, MMF], f32, tag=f"bk{t}", name=f"bk{t}")
                 for t in range(2)]

        # per-k bias columns for the ACT-built one-hot planes
        bias_k = {}
        for k in range(DVEK, KB):
            t = pool_misc.tile([P, 1], f32, tag=f"bk{k}")
            nc.gpsimd.memset(t[:], -float(k))
            bias_k[k] = t

        for c in range(NCHUNK):
            cs = slice(c * CF, (c + 1) * CF)
            pm16 = pool_in.tile([P, CF], bf16, tag="pm16")
            tm16 = pool_in.tile([P, CF], bf16, tag="tm16")
            ps16 = pool_in.tile([P, CF], bf16, tag="ps16")
            clsf = pool_in.tile([P, CF], f32, tag="clsf")
            nc.gpsimd.dma_start(out=pm16[:], in_=pm_v[:, cs])
            nc.gpsimd.dma_start(out=tm16[:], in_=tm_v[:, cs])
            nc.gpsimd.dma_start(out=ps16[:], in_=ps_v[:, cs])       # f32->bf16
            nc.sync.dma_start(out=clsf[:], in_=cls_v[:, cs])        # raw f32

            pohv = pool_poh.tile([P, MCOL * CF], bf16, tag="pohv")

            def col(i):
                return pohv[:, i * CF:(i + 1) * CF]

            # aux planes: ones, cls, ln(cls), ln(1-cls), ps^2.  The ln ops
            # MUST read f32 cls: bf16 rounds cls>0.998 to 1.0 -> ln(0).
            nc.gpsimd.memset(col(C_ONE), 1.0)
            nc.gpsimd.dma_start(out=col(C_CLS), in_=clsf[:])        # f32->bf16
            nc.scalar.activation(col(C_LNP), clsf[:], AF.Ln)
            nc.scalar.activation(col(C_LN1MP), clsf[:], AF.Ln,
                                 bias=1.0, scale=-1.0)
            nc.vector.tensor_tensor(col(C_PS2), ps16[:], ps16[:],
                                    mybir.AluOpType.mult)

            # pred one-hots: k < DVEK on DVE (is_equal @4x), rest on ACT via
            # relu(1-(pm-k)^2) -- exact for integer labels
            for k in range(DVEK):
                nc.vector.tensor_scalar(col(k), pm16[:], float(k), None, eq)
            for k in range(DVEK, KB):
                sq = pool_sq.tile([P, CF], bf16, tag="sq")
                nc.scalar.activation(sq[:], pm16[:], AF.Square, bias=bias_k[k][:])
                nc.scalar.activation(col(k), sq[:], AF.Relu, bias=1.0, scale=-1.0)

            pohv_kb = pohv[:].rearrange("p (m f) -> p m f", m=MCOL)

            for s in range(CF // SUBF):
                # target one-hots, 4-run interleaved layout [g, n, b]
                toh = pool_toh.tile([P, N * SUBF], bf16, tag="toh")
                toh4 = toh[:].rearrange("p (g n b) -> p g n b", n=N, b=GB)
                soff = s * SUBF
                for n in range(N):
                    nc.vector.tensor_scalar(
                        toh4[:, :, n, :],
                        tm16[:, soff:soff + SUBF], float(n), None, eq,
                    )
                for gg in range(NGH):
                    g = s * NGH + gg            # group within chunk
                    jg = c * NGRP + g           # global group index
                    t = jg % 2
                    nc.tensor.matmul(
                        banks[t][64 * t:64 * t + 64, :],
                        toh[:, 64 * gg:64 * gg + 64],
                        pohv_kb[:, :, GB * g:GB * g + GB],
                        start=(jg < 2),
                        stop=(jg >= NCHUNK * NGRP - 2),
                        tile_position=(0, 64 * t),
                        skip_group_check=True,
                    )

        # evacuate the two psum banks and store [128, 144] f32
        ev = pool_misc.tile([P, MMF], f32)
        nc.vector.tensor_copy(ev[0:64, :], banks[0][0:64, :])
        nc.vector.tensor_copy(ev[64:128, :], banks[1][64:128, :])
        nc.sync.dma_start(out=out_d[:, :], in_=ev[:])

    nc.finalize()
    return nc


def _get_nc():
    if "nc" not in _cached:
        _cached["nc"] = _build_bass()
    return _cached["nc"]


def _get_runner():
    """Build the sharded jitted executable ONCE; reuse across calls."""
    if "runner" in _cached:
        return _cached["runner"]

    import jax
    import concourse.mybir as mybir
    from jax.sharding import Mesh, PartitionSpec
    from jax.experimental.shard_map import shard_map
    from concourse import bass2jax

    bass2jax.install_neuronx_cc_hook()
    nc = _get_nc()
    partition_name = (
        nc.partition_id_tensor.name if nc.partition_id_tensor else None
    )

    in_names, out_names, out_avals, zero_outs = [], [], [], []
    for alloc in nc.m.functions[0].allocations:
        if not isinstance(alloc, mybir.MemoryLocationSet):
            continue
        name = alloc.memorylocations[0].name
        if alloc.kind == "ExternalInput":
            if name != partition_name:
                in_names.append(name)
        elif alloc.kind == "ExternalOutput":
            out_names.append(name)
            shape = tuple(alloc.tensor_shape)
            dtype = mybir.dt.np(alloc.dtype)
            out_avals.append(jax.core.ShapedArray(shape, dtype))
            zero_outs.append(np.zeros(shape, dtype))
    n_params = len(in_names)
    n_outs = len(out_avals)
    all_in_names = list(in_names) + list(out_names)
    if partition_name is not None:
        all_in_names.append(partition_name)
    donate = tuple(range(n_params, n_params + n_outs))

    def _body(*args):
        operands = list(args)
        if partition_name is not None:
            operands.append(bass2jax.partition_id_tensor())
        outs = bass2jax._bass_exec_p.bind(
            *operands,
            out_avals=tuple(out_avals),
            in_names=tuple(all_in_names),
            out_names=tuple(out_names),
            lowering_input_output_aliases=(),
            sim_require_finite=True,
            sim_require_nnan=True,
            nc=nc,
        )
        return tuple(outs)

    devices = jax.devices()[:NCORES]
    mesh = Mesh(np.asarray(devices), ("core",))
    in_specs = (PartitionSpec("core"),) * (n_params + n_outs)
    out_specs = (PartitionSpec("core"),) * n_outs
    sharded = jax.jit(
        shard_map(
            _body, mesh=mesh, in_specs=in_specs, out_specs=out_specs,
            check_rep=False,
        ),
        donate_argnums=donate,
        keep_unused=True,
    )

    def run(in_maps):
        if isinstance(in_maps, dict):
            # fast path: full flattened arrays (row-sharding = identity here)
            concat_in = [in_maps[name] for name in in_names]
        else:
            concat_in = [
                np.concatenate([np.asarray(m[name]) for m in in_maps], axis=0)
                for name in in_names
            ]
        concat_zeros = [
            np.zeros((NCORES * z.shape[0], *z.shape[1:]), z.dtype)
            for z in zero_outs
        ]
        out_arrs = sharded(*concat_in, *concat_zeros)
        return [
            {
                name: np.asarray(out_arrs[i]).reshape(
                    NCORES, *out_avals[i].shape)[c]
                for i, name in enumerate(out_names)
            }
            for c in range(NCORES)
        ]

    def bench(in_maps, iters=20):
        """Time the sharded call with device-resident inputs."""
        import time
        from jax.sharding import NamedSharding

        concat_in = [
            np.concatenate([np.asarray(m[name]) for m in in_maps], axis=0)
            for name in in_names
        ]
        shard = NamedSharding(mesh, PartitionSpec("core"))
        dev_in = [jax.device_put(x, shard) for x in concat_in]
        zeros = [
            np.zeros((NCORES * z.shape[0], *z.shape[1:]), z.dtype)
            for z in zero_outs
        ]

        def call():
            zs = [jax.device_put(z, shard) for z in zeros]
            outs = sharded(*dev_in, *zs)
            for o in outs:
                o.block_until_ready()

        call()
        ts = []
        for _ in range(iters):
            t0 = time.perf_counter()
            call()
            ts.append(time.perf_counter() - t0)
        return min(ts), sum(ts) / len(ts)

    run.bench = bench
    _cached["runner"] = run
    return run


def _reduce_core_out(ev):
    """[128, 144] f32 bank dump -> [16, 36] stats (diagonal extraction)."""
    acc = np.zeros((N, MCOL), dtype=np.float64)
    ev = ev.astype(np.float64)
    for t in range(2):
        rows = ev[64 * t:64 * t + 64].reshape(N, GB, MCOL, GB)  # [n, b, k, b']
        for b in range(GB):
            acc += rows[:, b, :, b]
    return acc


def kernel(pred_instance_mask, pred_score, cls_out, target_mask):
    run = _get_runner()

    # row-sharding across cores: the concatenation of per-core row slices is
    # just the full array flattened, so pass views of the full inputs
    full = {
        "pm": np.ascontiguousarray(pred_instance_mask).reshape(-1).astype(
            np.int32, copy=False),
        "tm": np.ascontiguousarray(target_mask).reshape(-1).astype(
            np.int32, copy=False),
        "cls": np.ascontiguousarray(cls_out).reshape(-1).astype(
            np.float32, copy=False),
        "ps": np.ascontiguousarray(pred_score).reshape(-1).astype(
            np.float32, copy=False),
    }

    outs = [r["out"] for r in run(full)]

    acc = np.zeros((N, MCOL), dtype=np.float64)
    for o in outs:
        acc += _reduce_core_out(o)
    return _host_finish(acc)


def _host_finish(acc):
    inter31 = acc[:, 0:KB]
    st = acc[:, C_ONE]                       # [N] target marginals
    inter = np.concatenate(
        [inter31, (st - inter31.sum(axis=1))[:, None]], axis=1)
    sp = inter.sum(axis=0)                   # [K] pred marginals

    sum_t = HWPIX - st[0]                    # count(target > 0)
    sum_p = acc[:, C_CLS].sum()              # sum(cls_out)
    sum_logp = acc[:, C_LNP].sum()
    inter_cls = sum_p - acc[0, C_CLS]        # sum over target>0 of cls_out
    bce_sum = (sum_logp - acc[0, C_LNP]) + acc[0, C_LN1MP]
    ps2 = acc[:, C_PS2].sum()

    mse = ps2 / HWPIX
    bce_cls = -bce_sum / HWPIX
    dice_cls = 1.0 - (2.0 * inter_cls + SMOOTH) / (sum_p + sum_t + SMOOTH)

    union = st[:, None] + sp[None, :]
    bce_pair = 100.0 * (union - 2.0 * inter) / HWPIX
    dice_pair = 1.0 - (2.0 * inter + SMOOTH) / (union + SMOOTH)
    pair = bce_pair + dice_pair
    res = mse + bce_cls + dice_cls + pair.min(axis=1).sum()
    return np.float32(res / float(N))
